# revision 29
# baseline (speedup 1.0000x reference)
"""Trainium2 Bass kernel for nn_Attention_14113262534866.

Self-attention over 64x64 "pixels" (n=4096), batch=2, heads=4, dim_head=32.
Sharding: one (batch, head) pair per NeuronCore (8 cores). Each core:
  - projects its head's q/k/v from x[b]  (1x1 conv == channel matmul)
  - computes softmax(q^T k / sqrt(d)) @ v in a transposed layout
    (dots^T[j, i] with j on partitions) so no attention transpose is needed
  - applies its head's slice of the output projection -> a partial [256, 4096]
Host unshard: sum the 4 head-partials per batch and reshape (bias is applied
on-device via an extra bias/4 row in the output projection).

The 16.8M-softmax-exp per core is the roofline; everything is organized
around feeding the two PSUM-capable elementwise engines:
  - QK dots are written to PSUM in BF16 (legal matmul output dtype; halves
    PSUM footprint) via 4x row-tiled K=32 strip matmuls -> groups of
    [128, 2048] spanning 2 banks.
  - ScalarE runs exact ACT exp on half the groups; VectorE runs a
    Schraudolph fast-exp on the rest (one fused mul+add tensor_scalar,
    bf16-in/int16-out = 2x_1P packed mode, output bitcast as bf16:
    u = round(x*128*log2e + 127*128 - 5.6) is the bf16 bit pattern of
    ~exp(x); the +-3.3% sawtooth error cancels in the softmax average).
    No max-subtraction is needed (dots ~ N(0,1)).
  - A@V uses a ones-augmented v^T and TWO col-tiled accumulation chains
    (tile_position (0,0)/(0,64), even/odd j-chunks, both start=True: psum
    has_written clears are per-region) so the AV matmuls run 2x concurrent.
  - The tail folds bias into augmented wo rows (32/96 = bias/4, riding the
    softmax-denominator rows) and normalizes once: s broadcast by a K=128
    matmul vs a 0/1 selector, 1/s via reciprocal_approx_fast, applied in
    the psum->sbuf move of the projected output. Output DMA'd in bf16.
QK and AV matmul emission is interleaved group-by-group so the PE's strict
in-order MM queue always has ready AV work behind each exp-gated QK group,
and the k/q projections + v^T transpose are interleaved into block 0 so
nothing waits on the tail of the x DMA.
"""

import ml_dtypes
import numpy as np

try:
    import concourse.mybir as mybir
except ImportError:  # concourse not on sys.path in this environment
    import sys
    for p in ("/opt/trn_rl_repo", "/root/.axon_site/_ro/trn_rl_repo"):
        if p not in sys.path:
            sys.path.insert(0, p)
    import concourse.mybir as mybir
import concourse.tile as tile
from concourse import bacc
from concourse.bass_utils import run_bass_kernel_spmd

F32 = mybir.dt.float32
BF16 = mybir.dt.bfloat16
I16 = mybir.dt.int16
EXP = mybir.ActivationFunctionType.Exp
MULT = mybir.AluOpType.mult
ADD = mybir.AluOpType.add

HEADS = 4
DIM_HEAD = 32
SCALE = DIM_HEAD ** -0.5
DIM = 256
N = 4096                 # 64*64 pixels
NB = 8                   # number of i-blocks
IB = 512                 # i-block width
JT = 32                  # j chunks of 128
NG = 16                  # QK groups per i-block (2 j-chunks each)
P = 128

# Schraudolph fast-exp constants (bf16 bit pattern via int16 round)
A_EXP = 128.0 * 1.4426950408889634
B_EXP = 127.0 * 128.0 - 5.6
# group index -> exp engine ('S'=ScalarE exact, 'V'=VectorE fast-exp)
GROUPS = [(2 * g, 2) for g in range(NG)]
V_SLOTS = (1, 3, 5, 7, 10, 12, 14)   # steady blocks
V_SLOTS_IB0 = (3, 7, 11)     # block 0 (VectorE also runs proj/vt copies)
# block-0 fillers emitted after each QK group: (k-tiles, q-tiles, vt-groups)
IB0_FILL = {0: ((1,), (1,), (0,)), 2: ((2,), (2,), (1,)),
            4: ((3,), (3,), (2,)), 6: ((4,), (4,), (3,)),
            8: ((5,), (5,), (4,)), 10: ((6,), (6,), (5,)),
            12: ((7,), (7,), (6,)), 13: ((), (), (7,))}


def build_program():
    nc = bacc.Bacc(None, target_bir_lowering=False, debug=False)

    x_d = nc.declare_dram_parameter("x", [2, P, N], BF16, isOutput=False)
    wq_d = nc.declare_dram_parameter("wq", [P, 2, 64], BF16, isOutput=False)
    wk_d = nc.declare_dram_parameter("wk", [P, 2, 64], BF16, isOutput=False)
    wv_d = nc.declare_dram_parameter("wv", [P, 2, 32], BF16, isOutput=False)
    wo_d = nc.declare_dram_parameter("wo", [P, 256], BF16, isOutput=False)
    out_d = nc.declare_dram_parameter("out", [DIM, N], BF16, isOutput=True)

    with tile.TileContext(nc) as tc:
        with (
            tc.tile_pool(name="const", bufs=1) as const,
            tc.tile_pool(name="qkv", bufs=1) as qkv,
            tc.tile_pool(name="attn", bufs=24) as attnp,
            tc.tile_pool(name="small", bufs=3) as small,
            tc.tile_pool(name="qk_ps", bufs=3, space="PSUM") as qk_ps,
            tc.tile_pool(name="av_ps", bufs=2, space="PSUM") as av_ps,
        ):
            # ---- constants / inputs to SBUF ----
            wq_sb = const.tile([P, 2, 64], BF16, tag="wq")
            wk_sb = const.tile([P, 2, 64], BF16, tag="wk")
            wv_sb = const.tile([P, 2, 32], BF16, tag="wv")
            wo_sb = const.tile([P, 256], BF16, tag="wo")
            nc.sync.dma_start(wk_sb[:], wk_d[:])
            nc.sync.dma_start(wq_sb[:], wq_d[:])
            x_sb = [const.tile([P, N], BF16, tag=f"x{c}", name=f"x_sb{c}")
                    for c in range(2)]
            X_CHUNKS = [(0, 512), (512, 1024), (1024, 2048), (2048, 3072),
                        (3072, 4096)]
            for c in range(2):
                lo, hi = X_CHUNKS[0]
                nc.sync.dma_start(x_sb[c][:, lo:hi], x_d[c][:, lo:hi])
            nc.sync.dma_start(wv_sb[:], wv_d[:])
            nc.sync.dma_start(wo_sb[:], wo_d[:])
            for lo, hi in X_CHUNKS[1:]:
                for c in range(2):
                    nc.sync.dma_start(x_sb[c][:, lo:hi], x_d[c][:, lo:hi])
            ones_f32 = const.tile([P, 1], F32, tag="ones_f32")
            nc.vector.memset(ones_f32[:], 1.0)
            # selector weights: rows 32/96 = 1 -> bcast(s_a + s_b)
            sel_sb = const.tile([P, P], BF16, tag="sel")
            nc.vector.memset(sel_sb[:], 0.0)
            nc.vector.memset(sel_sb[32:33, :], 1.0)
            nc.vector.memset(sel_sb[96:97, :], 1.0)
            # dummy exp so the ACT table set loads during setup, not at the
            # first real softmax tile
            act_warm = const.tile([P, 1], F32, tag="act_warm")
            nc.scalar.activation(act_warm[:], ones_f32[:], EXP)

            # q_rep/k_rep: [64, N] with the head's [32, N] duplicated on
            # partition groups (for 2x row-tiled QK strip matmuls)
            q_rep = qkv.tile([64, N], BF16, tag="q_rep")
            k_rep = qkv.tile([64, N], BF16, tag="k_rep")
            vT = qkv.tile([P, JT, 33], BF16, tag="vT")

            def proj_tile(dst, w_sb, t):
                # [64, 512] fp32 accumulation (K=256 over 2 c-chunks) in an
                # av-pool bank, then one bf16 copy out
                ps = av_ps.tile([P, IB], F32, tag="av", name="proj_ps_t")
                for c in range(2):
                    nc.tensor.matmul(
                        ps[0:64, :],
                        lhsT=w_sb[:, c, :],
                        rhs=x_sb[c][:, t * IB:(t + 1) * IB],
                        start=(c == 0), stop=(c == 1),
                    )
                nc.vector.tensor_copy(dst[:, t * IB:(t + 1) * IB],
                                      ps[0:64, :])

            def vt_group(gp):
                # vT[p, t, d] = v[d, 128t+p] for t in 4gp..4gp+3
                ps = av_ps.tile([P, IB], F32, tag="av", name="vt_ps_t")
                for lane in range(4):
                    pt = 4 * gp + lane
                    for c in range(2):
                        nc.tensor.matmul(
                            ps[:, 32 * lane:32 * lane + 32],
                            lhsT=x_sb[c][:, pt * P:(pt + 1) * P],
                            rhs=wv_sb[:, c, :],
                            start=(c == 0), stop=(c == 1),
                        )
                nc.vector.tensor_copy(
                    vT[:, 4 * gp:4 * gp + 4, 0:32],
                    ps[:, 0:P].rearrange("p (l d) -> p l d", l=4),
                )

            # ---- main attention loop ----
            attn_tiles = [[None] * NG for _ in range(NB)]

            def qk_group(ib, g, v_slots):
                base, sz = GROUPS[g]
                ps = qk_ps.tile([P, 2 * IB], F32, tag="qk", name="qk_ps_t")
                for half in range(sz):  # row-tiled (K=32, 3 strips)
                    jc = base + half
                    nc.tensor.matmul(
                        ps[:, half * IB:(half + 1) * IB],
                        lhsT=k_rep[32 * half:32 * half + 32,
                                   jc * P:(jc + 1) * P],
                        rhs=q_rep[32 * half:32 * half + 32,
                                  ib * IB:(ib + 1) * IB],
                        tile_position=(32 * half, 0),
                        start=True, stop=True,
                    )
                at = attnp.tile([P, 2 * IB], BF16, tag="attn", name="attn_t")
                if g in v_slots:
                    nc.vector.tensor_scalar(
                        at[:, 0:sz * IB].bitcast(I16), ps[:, 0:sz * IB],
                        A_EXP, B_EXP, MULT, ADD)
                else:
                    nc.scalar.activation(at[:, 0:sz * IB], ps[:, 0:sz * IB],
                                         EXP)
                attn_tiles[ib][g] = at

            def av_group(ib, g, av):
                # two col-tiled accumulation chains in one psum bank:
                # even j-chunks -> rows 0:33, odd -> rows 64:97; rows 32/96
                # are the softmax denominators (ones column of vT)
                base, sz = GROUPS[g]
                at = attn_tiles[ib][g]
                for half in range(sz):
                    jc = base + half
                    par = jc % 2
                    nc.tensor.matmul(
                        av[64 * par:64 * par + 33, :],
                        lhsT=vT[:, jc, :],
                        rhs=at[:, half * IB:(half + 1) * IB],
                        tile_position=(0, 64 * par),
                        start=(jc <= 1), stop=(jc >= 30),
                    )
                attn_tiles[ib][g] = None

            def tail_phase(ib, av):
                sb = small.tile([P, IB], BF16, tag="hout", name="hout_t")
                nc.vector.tensor_copy(sb[:], av[:])

                # tail psum: one borrowed qk tile = 3 independent banks for
                # bcast-s / pj0 / pj1 (no pool serialization)
                tl = qk_ps.tile([P, 2 * IB], F32, tag="qk", name="tail_ps_t")
                nc.tensor.matmul(tl[:, 0:IB], lhsT=sel_sb[:], rhs=sb[:],
                                 tile_position=(0, 0), start=True, stop=True)
                rcp = small.tile([P, IB], F32, tag="rcp", name="rcp_t")
                nc.vector.reciprocal_approx_fast(rcp[:], tl[:, 0:IB])
                tl2 = qk_ps.tile([P, 2 * IB], F32, tag="qk",
                                 name="tail2_ps_t")
                pjs = [tl[:, IB:2 * IB], tl2[:, 0:IB]]
                for ot in range(2):
                    pj = pjs[ot]
                    nc.tensor.matmul(pj,
                                     lhsT=wo_sb[:, ot * P:(ot + 1) * P],
                                     rhs=sb[:],
                                     tile_position=(0, 0),
                                     start=True, stop=True)
                    osb = small.tile([P, IB], BF16, tag=f"osb{ot}",
                                     name="osb_t")
                    nc.vector.tensor_mul(osb[:], pj, rcp[:])
                    nc.sync.dma_start(
                        out_d[ot * P:(ot + 1) * P, ib * IB:(ib + 1) * IB],
                        osb[:],
                    )

            # ---- emission ----
            # ones column of vT (col 32) early
            ones32_f32 = const.tile([P, JT], F32, tag="ones32")
            nc.vector.memset(ones32_f32[:], 1.0)
            nc.vector.tensor_copy(vT[:, :, 32], ones32_f32[:])

            # block 0: k-proj / q-proj tiles and vt groups are interleaved
            # between QK groups (IB0_FILL pacing) so each is emitted just
            # before its first consumer and never blocks the PE queue on
            # the x-DMA tail
            proj_tile(k_rep, wk_sb, 0)
            proj_tile(q_rep, wq_sb, 0)
            for g in range(NG):
                qk_group(0, g, V_SLOTS_IB0)
                ks, qs, vts = IB0_FILL.get(g, ((), (), ()))
                for t in ks:
                    proj_tile(k_rep, wk_sb, t)
                for t in qs:
                    proj_tile(q_rep, wq_sb, t)
                for gp in vts:
                    vt_group(gp)

            # zero the never-written av rows once per buffer (read by the
            # sel/wo matmuls with zero weights; must be finite)
            for b in range(2):
                av_init = av_ps.tile([P, IB], F32, tag="av",
                                     name=f"av_init{b}")
                nc.vector.memset(av_init[32:64, :], 0.0)
                nc.vector.memset(av_init[96:128, :], 0.0)

            for ib in range(1, NB + 1):
                av = av_ps.tile([P, IB], F32, tag="av", name="av_t")
                for g in range(NG):
                    if ib < NB:
                        qk_group(ib, g, V_SLOTS)
                    av_group(ib - 1, g, av)
                tail_phase(ib - 1, av)

    nc.compile()
    return nc


def make_core_inputs(x, w_qkv, w_out, b_out, core):
    b, h = core // HEADS, core % HEADS
    xb = np.ascontiguousarray(x[b].reshape(DIM, N)).astype(np.float32)
    w_q = w_qkv[h * 32:(h + 1) * 32, :] * SCALE
    w_k = w_qkv[128 + h * 32:128 + (h + 1) * 32, :]
    w_v = w_qkv[256 + h * 32:256 + (h + 1) * 32, :]
    wqT = np.ascontiguousarray(w_q.T)          # [256, 32]
    wkT = np.ascontiguousarray(w_k.T)
    wvT = np.ascontiguousarray(w_v.T)
    # layouts match SBUF tiles: [partition, c_chunk, m]; q/k duplicated
    # for the 2x row-tiled QK strips
    wq_in = np.stack([np.tile(wqT[c * P:(c + 1) * P], (1, 2))
                      for c in range(2)], axis=1)
    wk_in = np.stack([np.tile(wkT[c * P:(c + 1) * P], (1, 2))
                      for c in range(2)], axis=1)
    wv_in = np.stack([wvT[c * P:(c + 1) * P] for c in range(2)], axis=1)
    woT = np.ascontiguousarray(w_out[:, h * 32:(h + 1) * 32].T)  # [32, 256]
    wo_in = np.zeros((P, 256), np.float32)
    wo_in[0:32] = woT
    wo_in[64:96] = woT
    # bias rides the softmax-denominator rows: row32*s_a + row96*s_b then
    # * 1/s reconstructs bias/HEADS per core partial
    wo_in[32] = b_out / HEADS
    wo_in[96] = b_out / HEADS
    return {
        "x": xb.reshape(2, P, N).astype(ml_dtypes.bfloat16),
        "wq": wq_in.astype(ml_dtypes.bfloat16),
        "wk": wk_in.astype(ml_dtypes.bfloat16),
        "wv": wv_in.astype(ml_dtypes.bfloat16),
        "wo": wo_in.astype(ml_dtypes.bfloat16),
    }


_NC_CACHE = []


def get_nc():
    if not _NC_CACHE:
        _NC_CACHE.append(build_program())
    return _NC_CACHE[0]


def run(inputs, trace=False, tmpdir=None):
    nc = get_nc()
    in_maps = [
        make_core_inputs(inputs["x"], inputs["w_qkv"], inputs["w_out"],
                         inputs["b_out"], core)
        for core in range(8)
    ]
    kw = {}
    if trace:
        kw = dict(trace=True, tmpdir=tmpdir)
    res = run_bass_kernel_spmd(nc, in_maps, list(range(8)), **kw)
    b = inputs["x"].shape[0]
    hh, ww = inputs["x"].shape[2], inputs["x"].shape[3]
    out = np.zeros((b, DIM, hh, ww), np.float32)
    for bb in range(b):
        acc = np.zeros((DIM, N), np.float32)
        for h in range(HEADS):
            acc += res.results[bb * HEADS + h]["out"].astype(np.float32)
        out[bb] = acc.reshape(DIM, hh, ww)
    return out, res


def kernel(**inputs):
    out, _ = run(inputs)
    return out
